# revision 9
# baseline (speedup 1.0000x reference)
"""R-GCN (2-layer, basis-decomposed) forward pass on 8 Trainium2 NeuronCores.

Strategy
--------
Nodes are sharded contiguously across the 8 cores (12500 rows each). Each
relation's edge list is partitioned by destination shard and sorted by
destination node-tile (128 dst nodes per tile) on the host.

Key algebraic rearrangement: for each relation r,
    segment_sum(ew * (X @ V_r)[src], dst)  ==  segment_sum(ew * X[src], dst) @ V_r
so we aggregate raw features first (one gather of X rows per edge), then
apply the small basis-combined weight V_r once per 128-node output tile.

The bulk gather uses the custom InstDMAGatherAnt DMA (int16 indices), so the
node table is addressed as 4 sub-ranges of <=32768 rows; edges are grouped
by (tile, src-subrange, relation) and padded to whole 128-edge chunks.

Per 128-dst-node tile t, per relation r, per 128-edge chunk:
  - the tile's gathered source rows already sit in SBUF (bf16, one row per
    partition: chunk k edge e -> g[e, k, :]),
  - build the selection matrix M[e, n] = (dst_local[e] == n) * ew[e] with a
    single fused DVE tensor_scalar (iota row vs per-partition dst, then *ew),
  - PE matmul aggT[f, n] += Xg[e, f].T @ M[e, n], accumulated in PSUM.
Then h[n, :] = tanh(sum_r aggT_r.T @ V_r), stored as bf16. An AllGather
re-assembles the full [100000, 128] hidden table between the layers.
The classifier (h2 @ Wc + bc) is fused into the layer-2 tile epilogue.
"""

import math
import os
from contextlib import ExitStack
from dataclasses import dataclass

import numpy as np
import ml_dtypes

P = 128
BF16 = ml_dtypes.bfloat16


@dataclass
class Cfg:
    n_nodes: int = 100000       # total nodes
    n_cores: int = 8
    feat: int = 128             # F == H == 128 (one partition dim)
    n_rel: int = 4
    n_classes: int = 3
    n_sub: int = 4              # node-table subranges (int16 gather indices)

    @property
    def shard(self) -> int:
        assert self.n_nodes % self.n_cores == 0
        return self.n_nodes // self.n_cores

    @property
    def n_tiles(self) -> int:
        return math.ceil(self.shard / P)

    @property
    def last_rows(self) -> int:
        return self.shard - (self.n_tiles - 1) * P

    @property
    def sub_size(self) -> int:
        assert self.n_nodes % self.n_sub == 0
        s = self.n_nodes // self.n_sub
        assert s <= 32767
        return s


@dataclass
class Plan:
    K: np.ndarray           # [n_tiles, n_sub, n_rel] chunk counts (uniform over cores)
    tile_col: np.ndarray    # [n_tiles + 1] chunk-column offset of each tile
    seg: list               # seg[t][s] = (col0, ncols) gather segment
    rcols: list             # rcols[t][r] = list of chunk columns for relation r

    @property
    def C_total(self) -> int:
        return int(self.tile_col[-1])


def make_plan(cfg: Cfg, counts):
    """counts: [n_cores, n_tiles, n_sub, n_rel] edge counts."""
    K = np.ceil(counts.max(axis=0) / P).astype(np.int64)  # [T, S, R]
    # make sure every (t, r) has at least one chunk so PSUM groups exist
    for t in range(cfg.n_tiles):
        for r in range(cfg.n_rel):
            if K[t, :, r].sum() == 0:
                K[t, 0, r] = 1
    tile_col = np.zeros((cfg.n_tiles + 1,), np.int64)
    seg = []
    rcols = []
    for t in range(cfg.n_tiles):
        col = int(tile_col[t])
        segs_t = []
        rc_t = [[] for _ in range(cfg.n_rel)]
        for s in range(cfg.n_sub):
            ncols = int(K[t, s].sum())
            segs_t.append((col, ncols))
            for r in range(cfg.n_rel):
                for _ in range(int(K[t, s, r])):
                    rc_t[r].append(col)
                    col += 1
        tile_col[t + 1] = col
        seg.append(segs_t)
        rcols.append(rc_t)
    return Plan(K=K, tile_col=tile_col, seg=seg, rcols=rcols)


def preprocess_edges(cfg: Cfg, src, dst, ew):
    """Partition/sort/pad edges; build per-core packed arrays.

    Returns (plan, per_core) with per_core[c] = dict(idx16, dst_f32, ew_f32).
    """
    R, S, ncore = cfg.n_rel, cfg.n_sub, cfg.n_cores
    shard, ntiles, sub = cfg.shard, cfg.n_tiles, cfg.sub_size
    buckets = [[None] * R for _ in range(ncore)]
    counts = np.zeros((ncore, ntiles, S, R), np.int64)
    for r in range(R):
        d = np.asarray(dst[r]).astype(np.int64)
        s_arr = np.asarray(src[r]).astype(np.int64)
        w = np.asarray(ew[r]).astype(np.float32)
        core = d // shard
        for c in range(ncore):
            sel = np.nonzero(core == c)[0]
            dl = d[sel] - c * shard
            t = dl // P
            sb = s_arr[sel] // sub
            # sort by (tile, subtable)
            order = np.lexsort((sb, t))
            sel = sel[order]
            dl = dl[order]
            t = t[order]
            sb = sb[order]
            np.add.at(counts[c], (t, sb, np.full_like(t, r)), 1)
            buckets[c][r] = (
                (s_arr[sel] % sub).astype(np.int16),
                (dl % P).astype(np.float32),
                w[sel],
                t * S + sb,  # group id within relation
            )
    plan = make_plan(cfg, counts)
    C = plan.C_total

    per_core = []
    for c in range(ncore):
        src_fl = np.zeros((C * P,), np.int16)
        dst_fl = np.zeros((C * P,), np.float32)
        ew_fl = np.zeros((C * P,), np.float32)
        # group start positions per (r): cumulative counts over (t, s)
        for r in range(R):
            s16, dl, w, gid = buckets[c][r]
            gstarts = np.zeros((ntiles * S + 1,), np.int64)
            np.add.at(gstarts, gid + 1, 1)
            gstarts = np.cumsum(gstarts)
            # slot base for each group (t, s) for this relation
            for t in range(ntiles):
                for s in range(S):
                    a, b = gstarts[t * S + s], gstarts[t * S + s + 1]
                    n = int(b - a)
                    if n == 0:
                        continue
                    kk = int(plan.K[t, s, r])
                    assert n <= kk * P, (c, r, t, s, n, kk)
                    # column of first chunk of (t, s, r):
                    col0 = plan.seg[t][s][0] + int(plan.K[t, s, :r].sum())
                    base = col0 * P
                    src_fl[base : base + n] = s16[a:b]
                    dst_fl[base : base + n] = dl[a:b]
                    ew_fl[base : base + n] = w[a:b]
        # chunk-column layouts
        dst_pk = np.ascontiguousarray(dst_fl.reshape(C, P).T)
        ew_pk = np.ascontiguousarray(ew_fl.reshape(C, P).T)
        # idx16 wrapped layout: slot j -> [j%16, j//16], replicated to 128 rows
        idx16 = np.tile(src_fl.reshape(C * 8, 16).T, (8, 1))
        per_core.append(
            dict(
                idx16=np.ascontiguousarray(idx16),
                dst_f32=dst_pk,
                ew_f32=ew_pk,
            )
        )
    return plan, per_core


def build_program(cfg: Cfg, plan: Plan, enable_asserts=False):
    """Build the SPMD Bass program (identical on all cores)."""
    from concourse import bacc, bass, mybir
    from concourse import tile as tile_mod

    dt = mybir.dt
    R, S, ntiles, shard, NC = cfg.n_rel, cfg.n_sub, cfg.n_tiles, cfg.shard, cfg.n_classes
    C = plan.C_total
    sub = cfg.sub_size

    nc = bacc.Bacc(
        "TRN2",
        target_bir_lowering=False,
        debug=False,
        enable_asserts=enable_asserts,
        num_devices=cfg.n_cores,
    )

    x1 = nc.dram_tensor("x1", [cfg.n_nodes, P], dt.bfloat16, kind="ExternalInput")
    idxp = nc.dram_tensor("idxp", [P, C * 8], dt.int16, kind="ExternalInput")
    dstp = nc.dram_tensor("dstp", [P, C], dt.float32, kind="ExternalInput")
    ewp = nc.dram_tensor("ewp", [P, C], dt.float32, kind="ExternalInput")
    v1 = nc.dram_tensor("v1", [R, P, P], dt.bfloat16, kind="ExternalInput")
    v2 = nc.dram_tensor("v2", [R, P, P], dt.bfloat16, kind="ExternalInput")
    wc = nc.dram_tensor("wc", [P, NC], dt.bfloat16, kind="ExternalInput")
    bcb = nc.dram_tensor("bcb", [P, NC], dt.float32, kind="ExternalInput")
    out = nc.dram_tensor("out", [shard, NC], dt.float32, kind="ExternalOutput")

    max_tile_cols = int((plan.tile_col[1:] - plan.tile_col[:-1]).max())

    with ExitStack() as ctx:
        tc = ctx.enter_context(tile_mod.TileContext(nc))
        const = ctx.enter_context(tc.tile_pool(name="const", bufs=1))
        meta = ctx.enter_context(tc.tile_pool(name="meta", bufs=1))
        idxpool = ctx.enter_context(tc.tile_pool(name="idxp", bufs=3))
        gat = ctx.enter_context(tc.tile_pool(name="gat", bufs=2))
        msb = ctx.enter_context(tc.tile_pool(name="msb", bufs=6))
        asb = ctx.enter_context(tc.tile_pool(name="asb", bufs=2))
        hsb = ctx.enter_context(tc.tile_pool(name="hsb", bufs=3))
        psA = ctx.enter_context(tc.tile_pool(name="psA", bufs=3, space="PSUM"))
        psB = ctx.enter_context(tc.tile_pool(name="psB", bufs=2, space="PSUM"))
        dram = ctx.enter_context(tc.tile_pool(name="dram", bufs=1, space="DRAM"))

        # constants
        iota32 = const.tile([P, P], dt.int32)
        nc.gpsimd.iota(iota32[:], pattern=[[1, P]], base=0, channel_multiplier=0)
        iotab = const.tile([P, P], dt.bfloat16)
        nc.vector.tensor_copy(iotab[:], iota32[:])
        v1sb = const.tile([P, R * P], dt.bfloat16)
        v2sb = const.tile([P, R * P], dt.bfloat16)
        for r in range(R):
            nc.sync.dma_start(v1sb[:, r * P : (r + 1) * P], v1[r, :, :])
            nc.sync.dma_start(v2sb[:, r * P : (r + 1) * P], v2[r, :, :])
        wcsb = const.tile([P, NC], dt.bfloat16)
        nc.sync.dma_start(wcsb[:], wc[:, :])
        bcsb = const.tile([P, NC], dt.float32)
        nc.sync.dma_start(bcsb[:], bcb[:, :])

        # per-chunk metadata (shared by both layers)
        dst_sb = meta.tile([P, C], dt.float32)
        ew_sb = meta.tile([P, C], dt.float32)
        nc.sync.dma_start(dst_sb[:], dstp[:, :])
        nc.sync.dma_start(ew_sb[:], ewp[:, :])

        h1s = dram.tile([shard, P], dt.bfloat16)
        h1f = dram.tile([cfg.n_nodes, P], dt.bfloat16)

        # one register per distinct gather count (fresh reg_movs per call
        # blow up register allocation at ~800 gather instructions)
        reg_cache = {}

        def nreg(v):
            if v not in reg_cache:
                reg_cache[v] = nc.gpsimd.to_reg(v)
            return reg_cache[v]

        def layer(x_table, vsb, last):
            for t in range(ntiles):
                c0 = int(plan.tile_col[t])
                Kt = int(plan.tile_col[t + 1] - plan.tile_col[t])
                # stage this tile's gather indices
                i_sb = idxpool.tile([P, max_tile_cols * 8], dt.int16, tag="idx")
                nc.sync.dma_start(
                    i_sb[:, : Kt * 8], idxp[:, c0 * 8 : (c0 + Kt) * 8]
                )
                g = gat.tile([P, max_tile_cols * P], dt.bfloat16, tag="g")
                g3 = g[:].rearrange("p (k f) -> p k f", f=P)
                for s in range(S):
                    scol, ncols = plan.seg[t][s]
                    if ncols == 0:
                        continue
                    k0 = scol - c0
                    nc.gpsimd.dma_gather(
                        out_ap=g3[:, k0 : k0 + ncols, :],
                        in_ap=x_table[s * sub : (s + 1) * sub, :],
                        idxs_ap=i_sb[:, k0 * 8 : (k0 + ncols) * 8],
                        num_idxs=ncols * P,
                        num_idxs_reg=nreg(ncols * P),
                        elem_size=P,
                        single_packet=False,
                    )
                aggs = []
                for r in range(R):
                    cols = plan.rcols[t][r]
                    ps = psA.tile([P, P], dt.float32, tag="aggT")
                    for k, cc in enumerate(cols):
                        m = msb.tile([P, P], dt.bfloat16, tag="m")
                        nc.vector.tensor_scalar(
                            out=m[:],
                            in0=iotab[:],
                            scalar1=dst_sb[:, cc : cc + 1],
                            scalar2=ew_sb[:, cc : cc + 1],
                            op0=mybir.AluOpType.is_equal,
                            op1=mybir.AluOpType.mult,
                        )
                        kk = cc - c0
                        nc.tensor.matmul(
                            ps[:],
                            lhsT=g[:, kk * P : (kk + 1) * P],
                            rhs=m[:],
                            start=(k == 0),
                            stop=(k == len(cols) - 1),
                        )
                    a_sb = asb.tile([P, P], dt.bfloat16, tag=f"agg{r}")
                    nc.vector.tensor_copy(a_sb[:], ps[:])
                    aggs.append(a_sb)
                rows = cfg.last_rows if t == ntiles - 1 else P
                ps2 = psB.tile([P, P], dt.float32, tag="o2")
                if not last:
                    # h[n, h'] = tanh(sum_r agg_r[n, :] @ V_r)
                    for r in range(R):
                        nc.tensor.matmul(
                            ps2[:],
                            lhsT=aggs[r][:],
                            rhs=vsb[:, r * P : (r + 1) * P],
                            start=(r == 0),
                            stop=(r == R - 1),
                        )
                    h = hsb.tile([P, P], dt.bfloat16, tag="h")
                    nc.scalar.activation(
                        h[:], ps2[:], mybir.ActivationFunctionType.Tanh
                    )
                    nc.sync.dma_start(h1s[t * P : t * P + rows, :], h[:rows, :])
                else:
                    # transposed: h2T[h', n] = tanh(sum_r V_r.T @ aggT_r)
                    for r in range(R):
                        nc.tensor.matmul(
                            ps2[:],
                            lhsT=vsb[:, r * P : (r + 1) * P],
                            rhs=aggs[r][:],
                            start=(r == 0),
                            stop=(r == R - 1),
                        )
                    h2t = hsb.tile([P, P], dt.bfloat16, tag="h2t")
                    nc.scalar.activation(
                        h2t[:], ps2[:], mybir.ActivationFunctionType.Tanh
                    )
                    ps3 = psB.tile([P, NC], dt.float32, tag="cls")
                    nc.tensor.matmul(
                        ps3[:], lhsT=h2t[:], rhs=wcsb[:], start=True, stop=True
                    )
                    o = hsb.tile([P, NC], dt.float32, tag="o")
                    nc.vector.tensor_tensor(
                        out=o[:], in0=ps3[:], in1=bcsb[:], op=mybir.AluOpType.add
                    )
                    nc.sync.dma_start(out[t * P : t * P + rows, :], o[:rows, :])

        layer(x1[:, :], v1sb, last=False)
        nc.gpsimd.collective_compute(
            "AllGather",
            mybir.AluOpType.bypass,
            replica_groups=[list(range(cfg.n_cores))],
            ins=[h1s.opt()],
            outs=[h1f.opt()],
        )
        layer(h1f[:, :], v2sb, last=True)

    nc.compile()
    return nc


def make_in_maps(cfg: Cfg, per_core, emb, W1, W1_comp, W2, W2_comp, Wc, bc):
    V1 = np.einsum("rb,bio->rio", W1_comp, W1).astype(BF16)
    V2 = np.einsum("rb,bio->rio", W2_comp, W2).astype(BF16)
    x1 = np.ascontiguousarray(np.asarray(emb).astype(BF16))
    wc = np.asarray(Wc).astype(BF16)
    bcb = np.tile(np.asarray(bc).astype(np.float32)[None, :], (P, 1))
    in_maps = []
    for c in range(cfg.n_cores):
        pc = per_core[c]
        in_maps.append(
            dict(
                x1=x1,
                idxp=pc["idx16"],
                dstp=pc["dst_f32"],
                ewp=pc["ew_f32"],
                v1=V1,
                v2=V2,
                wc=wc,
                bcb=bcb,
            )
        )
    return in_maps


def run_program(nc, cfg: Cfg, in_maps, trace=False, tmpdir=None):
    from concourse import bass_utils

    res = bass_utils.run_bass_kernel_spmd(
        nc,
        in_maps,
        core_ids=list(range(cfg.n_cores)),
        trace=trace,
        tmpdir=tmpdir,
    )
    outs = [res.results[c]["out"] for c in range(cfg.n_cores)]
    return np.concatenate(outs, axis=0), res


def kernel(emb, W1, W1_comp, W2, W2_comp, Wc, bc, ew, src, dst):
    cfg = Cfg()
    plan, per_core = preprocess_edges(cfg, src, dst, ew)
    nc = build_program(cfg, plan)
    in_maps = make_in_maps(cfg, per_core, emb, W1, W1_comp, W2, W2_comp, Wc, bc)
    out, _ = run_program(nc, cfg, in_maps, trace=False)
    return out.astype(np.float32)


# revision 12
# speedup vs baseline: 2.8541x; 2.8541x over previous
"""R-GCN (2-layer, basis-decomposed) forward pass on 8 Trainium2 NeuronCores.

Strategy
--------
Nodes are sharded contiguously across the 8 cores (12500 rows each). Each
relation's edge list is partitioned by destination shard and sorted by
destination node-tile (128 dst nodes per tile) on the host.

Key algebraic rearrangement: for each relation r,
    segment_sum(ew * (X @ V_r)[src], dst)  ==  segment_sum(ew * X[src], dst) @ V_r
so we aggregate raw features first (one gather of X rows per edge), then
apply the small basis-combined weight V_r once per 128-node output tile.

The bulk gather uses the custom InstDMAGatherAnt DMA (int16 indices), so the
node table is addressed as 4 sub-ranges of <=32768 rows; edges are grouped
by (tile, src-subrange, relation) and padded to whole 128-edge chunks.

Per 128-dst-node tile t, per relation r, per 128-edge chunk:
  - the tile's gathered source rows already sit in SBUF (bf16, one row per
    partition: chunk k edge e -> g[e, k, :]),
  - build the selection matrix M[e, n] = (dst_local[e] == n) * ew[e] with a
    single fused DVE tensor_scalar (iota row vs per-partition dst, then *ew),
  - PE matmul aggT[f, n] += Xg[e, f].T @ M[e, n], accumulated in PSUM.
Then h[n, :] = tanh(sum_r aggT_r.T @ V_r), stored as bf16. An AllGather
re-assembles the full [100000, 128] hidden table between the layers.
The classifier (h2 @ Wc + bc) is fused into the layer-2 tile epilogue.
"""

import math
import os
from contextlib import ExitStack
from dataclasses import dataclass

import numpy as np
import ml_dtypes

P = 128
BF16 = ml_dtypes.bfloat16


@dataclass
class Cfg:
    n_nodes: int = 100000       # total nodes
    n_cores: int = 8
    feat: int = 128             # F == H == 128 (one partition dim)
    n_rel: int = 4
    n_classes: int = 3
    n_sub: int = 4              # node-table subranges (int16 gather indices)

    @property
    def shard(self) -> int:
        assert self.n_nodes % self.n_cores == 0
        return self.n_nodes // self.n_cores

    @property
    def n_tiles(self) -> int:
        return math.ceil(self.shard / P)

    @property
    def last_rows(self) -> int:
        return self.shard - (self.n_tiles - 1) * P

    @property
    def sub_size(self) -> int:
        assert self.n_nodes % self.n_sub == 0
        s = self.n_nodes // self.n_sub
        assert s <= 32767
        return s


@dataclass
class Plan:
    K: np.ndarray           # [n_tiles, n_sub, n_rel] chunk counts (uniform over cores)
    tile_col: np.ndarray    # [n_tiles + 1] chunk-column offset of each tile
    seg: list               # seg[t][s] = (col0, ncols) gather segment
    rcols: list             # rcols[t][r] = list of chunk columns for relation r

    @property
    def C_total(self) -> int:
        return int(self.tile_col[-1])


def make_plan(cfg: Cfg, counts):
    """counts: [n_cores, n_tiles, n_sub, n_rel] edge counts."""
    K = np.ceil(counts.max(axis=0) / P).astype(np.int64)  # [T, S, R]
    # make sure every (t, r) has at least one chunk so PSUM groups exist
    for t in range(cfg.n_tiles):
        for r in range(cfg.n_rel):
            if K[t, :, r].sum() == 0:
                K[t, 0, r] = 1
    tile_col = np.zeros((cfg.n_tiles + 1,), np.int64)
    seg = []
    rcols = []
    for t in range(cfg.n_tiles):
        col = int(tile_col[t])
        segs_t = []
        rc_t = [[] for _ in range(cfg.n_rel)]
        for s in range(cfg.n_sub):
            ncols = int(K[t, s].sum())
            segs_t.append((col, ncols))
            for r in range(cfg.n_rel):
                for _ in range(int(K[t, s, r])):
                    rc_t[r].append(col)
                    col += 1
        tile_col[t + 1] = col
        seg.append(segs_t)
        rcols.append(rc_t)
    return Plan(K=K, tile_col=tile_col, seg=seg, rcols=rcols)


def preprocess_edges(cfg: Cfg, src, dst, ew):
    """Partition/sort/pad edges; build per-core packed arrays.

    Returns (plan, per_core) with per_core[c] = dict(idx16, dst_b16, ew_b16).
    """
    R, S, ncore = cfg.n_rel, cfg.n_sub, cfg.n_cores
    shard, ntiles, sub = cfg.shard, cfg.n_tiles, cfg.sub_size
    buckets = [[None] * R for _ in range(ncore)]
    counts = np.zeros((ncore, ntiles, S, R), np.int64)
    for r in range(R):
        d = np.asarray(dst[r]).astype(np.int64)
        s_arr = np.asarray(src[r]).astype(np.int64)
        w = np.asarray(ew[r]).astype(np.float32)
        core = d // shard
        for c in range(ncore):
            sel = np.nonzero(core == c)[0]
            dl = d[sel] - c * shard
            t = dl // P
            sb = s_arr[sel] // sub
            # sort by (tile, subtable)
            order = np.lexsort((sb, t))
            sel = sel[order]
            dl = dl[order]
            t = t[order]
            sb = sb[order]
            np.add.at(counts[c], (t, sb, np.full_like(t, r)), 1)
            buckets[c][r] = (
                (s_arr[sel] % sub).astype(np.int16),
                (dl % P).astype(np.float32),
                w[sel],
                t * S + sb,  # group id within relation
            )
    plan = make_plan(cfg, counts)
    C = plan.C_total

    per_core = []
    for c in range(ncore):
        src_fl = np.zeros((C * P,), np.int16)
        dst_fl = np.zeros((C * P,), np.float32)
        ew_fl = np.zeros((C * P,), np.float32)
        # group start positions per (r): cumulative counts over (t, s)
        for r in range(R):
            s16, dl, w, gid = buckets[c][r]
            gstarts = np.zeros((ntiles * S + 1,), np.int64)
            np.add.at(gstarts, gid + 1, 1)
            gstarts = np.cumsum(gstarts)
            # slot base for each group (t, s) for this relation
            for t in range(ntiles):
                for s in range(S):
                    a, b = gstarts[t * S + s], gstarts[t * S + s + 1]
                    n = int(b - a)
                    if n == 0:
                        continue
                    kk = int(plan.K[t, s, r])
                    assert n <= kk * P, (c, r, t, s, n, kk)
                    # column of first chunk of (t, s, r):
                    col0 = plan.seg[t][s][0] + int(plan.K[t, s, :r].sum())
                    base = col0 * P
                    src_fl[base : base + n] = s16[a:b]
                    dst_fl[base : base + n] = dl[a:b]
                    ew_fl[base : base + n] = w[a:b]
        # chunk-column layouts (bf16: dst values <=127 exact; ew rounded)
        dst_pk = np.ascontiguousarray(dst_fl.reshape(C, P).T.astype(BF16))
        ew_pk = np.ascontiguousarray(ew_fl.reshape(C, P).T.astype(BF16))
        # idx16 wrapped layout: slot j -> [j%16, j//16], replicated to 128 rows
        idx16 = np.tile(src_fl.reshape(C * 8, 16).T, (8, 1))
        per_core.append(
            dict(
                idx16=np.ascontiguousarray(idx16),
                dst_b16=dst_pk,
                ew_b16=ew_pk,
            )
        )
    return plan, per_core


def build_program(cfg: Cfg, plan: Plan, enable_asserts=False):
    """Build the SPMD Bass program (identical on all cores)."""
    from concourse import bacc, bass, mybir
    from concourse import tile as tile_mod

    dt = mybir.dt
    R, S, ntiles, shard, NC = cfg.n_rel, cfg.n_sub, cfg.n_tiles, cfg.shard, cfg.n_classes
    C = plan.C_total
    sub = cfg.sub_size

    nc = bacc.Bacc(
        "TRN2",
        target_bir_lowering=False,
        debug=False,
        enable_asserts=enable_asserts,
        num_devices=cfg.n_cores,
        num_swdge_queues=4,
    )

    x1 = nc.dram_tensor("x1", [cfg.n_nodes, P], dt.bfloat16, kind="ExternalInput")
    idxp = nc.dram_tensor("idxp", [P, C * 8], dt.int16, kind="ExternalInput")
    dstp = nc.dram_tensor("dstp", [P, C], dt.bfloat16, kind="ExternalInput")
    ewp = nc.dram_tensor("ewp", [P, C], dt.bfloat16, kind="ExternalInput")
    v1 = nc.dram_tensor("v1", [R, P, P], dt.bfloat16, kind="ExternalInput")
    v2 = nc.dram_tensor("v2", [R, P, P], dt.bfloat16, kind="ExternalInput")
    wc = nc.dram_tensor("wc", [P, NC], dt.bfloat16, kind="ExternalInput")
    bcb = nc.dram_tensor("bcb", [P, NC], dt.float32, kind="ExternalInput")
    out = nc.dram_tensor("out", [shard, NC], dt.float32, kind="ExternalOutput")

    max_tile_cols = int((plan.tile_col[1:] - plan.tile_col[:-1]).max())

    with ExitStack() as ctx:
        tc = ctx.enter_context(tile_mod.TileContext(nc))
        const = ctx.enter_context(tc.tile_pool(name="const", bufs=1))
        meta = ctx.enter_context(tc.tile_pool(name="meta", bufs=1))
        idxpool = ctx.enter_context(tc.tile_pool(name="idxp", bufs=3))
        gat = ctx.enter_context(tc.tile_pool(name="gat", bufs=2))
        msb = ctx.enter_context(tc.tile_pool(name="msb", bufs=6))
        asb = ctx.enter_context(tc.tile_pool(name="asb", bufs=2))
        hsb = ctx.enter_context(tc.tile_pool(name="hsb", bufs=3))
        psA = ctx.enter_context(tc.tile_pool(name="psA", bufs=3, space="PSUM"))
        psB = ctx.enter_context(tc.tile_pool(name="psB", bufs=2, space="PSUM"))
        dram = ctx.enter_context(tc.tile_pool(name="dram", bufs=1, space="DRAM"))

        # constants
        iota32 = const.tile([P, P], dt.int32)
        nc.gpsimd.iota(iota32[:], pattern=[[1, P]], base=0, channel_multiplier=0)
        iotab = const.tile([P, P], dt.bfloat16)
        nc.vector.tensor_copy(iotab[:], iota32[:])
        v1sb = const.tile([P, R * P], dt.bfloat16)
        v2sb = const.tile([P, R * P], dt.bfloat16)
        for r in range(R):
            nc.sync.dma_start(v1sb[:, r * P : (r + 1) * P], v1[r, :, :])
            nc.sync.dma_start(v2sb[:, r * P : (r + 1) * P], v2[r, :, :])
        wcsb = const.tile([P, NC], dt.bfloat16)
        nc.sync.dma_start(wcsb[:], wc[:, :])
        bcsb = const.tile([P, NC], dt.float32)
        nc.sync.dma_start(bcsb[:], bcb[:, :])

        # per-chunk metadata (shared by both layers)
        dst_sb = meta.tile([P, C], dt.bfloat16)
        ew_sb = meta.tile([P, C], dt.bfloat16)
        nc.sync.dma_start(dst_sb[:], dstp[:, :])
        nc.sync.dma_start(ew_sb[:], ewp[:, :])

        h1s = dram.tile([shard, P], dt.bfloat16)
        h1f = dram.tile([cfg.n_nodes, P], dt.bfloat16)

        # one register per distinct gather count (fresh reg_movs per call
        # blow up register allocation at ~800 gather instructions)
        reg_cache = {}

        def nreg(v):
            if v not in reg_cache:
                reg_cache[v] = nc.gpsimd.to_reg(v)
            return reg_cache[v]

        def layer(x_table, vsb, last):
            for t in range(ntiles):
                c0 = int(plan.tile_col[t])
                Kt = int(plan.tile_col[t + 1] - plan.tile_col[t])
                # stage this tile's gather indices
                i_sb = idxpool.tile([P, max_tile_cols * 8], dt.int16, tag="idx")
                nc.sync.dma_start(
                    i_sb[:, : Kt * 8], idxp[:, c0 * 8 : (c0 + Kt) * 8]
                )
                g = gat.tile([P, max_tile_cols * P], dt.bfloat16, tag="g")
                g3 = g[:].rearrange("p (k f) -> p k f", f=P)
                for s in range(S):
                    scol, ncols = plan.seg[t][s]
                    if ncols == 0:
                        continue
                    k0 = scol - c0
                    nc.gpsimd.dma_gather(
                        out_ap=g3[:, k0 : k0 + ncols, :],
                        in_ap=x_table[s * sub : (s + 1) * sub, :],
                        idxs_ap=i_sb[:, k0 * 8 : (k0 + ncols) * 8],
                        num_idxs=ncols * P,
                        num_idxs_reg=nreg(ncols * P),
                        elem_size=P,
                        single_packet=False,
                        queue_num=s % 4,
                    )
                # tile-wide selection matrix: M[e, kk*P + n] =
                #   (dst_local[e,chunk kk] == n) * ew[e,chunk kk]
                mt = msb.tile([P, max_tile_cols * P], dt.bfloat16, tag="m")
                m3 = mt[:, : Kt * P].rearrange("p (k f) -> p k f", f=P)
                iota3 = (
                    iotab[:]
                    .rearrange("p (k f) -> p k f", k=1)
                    .to_broadcast([P, Kt, P])
                )
                dst3 = (
                    dst_sb[:, c0 : c0 + Kt]
                    .rearrange("p (k f) -> p k f", f=1)
                    .to_broadcast([P, Kt, P])
                )
                ew3 = (
                    ew_sb[:, c0 : c0 + Kt]
                    .rearrange("p (k f) -> p k f", f=1)
                    .to_broadcast([P, Kt, P])
                )
                nc.vector.tensor_tensor(
                    out=m3, in0=iota3, in1=dst3, op=mybir.AluOpType.is_equal
                )
                nc.vector.tensor_tensor(
                    out=m3, in0=m3, in1=ew3, op=mybir.AluOpType.mult
                )
                aggs = []
                for r in range(R):
                    cols = plan.rcols[t][r]
                    ps = psA.tile([P, P], dt.float32, tag="aggT")
                    for k, cc in enumerate(cols):
                        kk = cc - c0
                        nc.tensor.matmul(
                            ps[:],
                            lhsT=g[:, kk * P : (kk + 1) * P],
                            rhs=mt[:, kk * P : (kk + 1) * P],
                            start=(k == 0),
                            stop=(k == len(cols) - 1),
                        )
                    a_sb = asb.tile([P, P], dt.bfloat16, tag=f"agg{r}")
                    nc.scalar.activation(
                        a_sb[:], ps[:], mybir.ActivationFunctionType.Copy
                    )
                    aggs.append(a_sb)
                rows = cfg.last_rows if t == ntiles - 1 else P
                ps2 = psB.tile([P, P], dt.float32, tag="o2")
                if not last:
                    # h[n, h'] = tanh(sum_r agg_r[n, :] @ V_r)
                    for r in range(R):
                        nc.tensor.matmul(
                            ps2[:],
                            lhsT=aggs[r][:],
                            rhs=vsb[:, r * P : (r + 1) * P],
                            start=(r == 0),
                            stop=(r == R - 1),
                        )
                    h = hsb.tile([P, P], dt.bfloat16, tag="h")
                    nc.scalar.activation(
                        h[:], ps2[:], mybir.ActivationFunctionType.Tanh
                    )
                    nc.sync.dma_start(h1s[t * P : t * P + rows, :], h[:rows, :])
                else:
                    # transposed: h2T[h', n] = tanh(sum_r V_r.T @ aggT_r)
                    for r in range(R):
                        nc.tensor.matmul(
                            ps2[:],
                            lhsT=vsb[:, r * P : (r + 1) * P],
                            rhs=aggs[r][:],
                            start=(r == 0),
                            stop=(r == R - 1),
                        )
                    h2t = hsb.tile([P, P], dt.bfloat16, tag="h2t")
                    nc.scalar.activation(
                        h2t[:], ps2[:], mybir.ActivationFunctionType.Tanh
                    )
                    ps3 = psB.tile([P, NC], dt.float32, tag="cls")
                    nc.tensor.matmul(
                        ps3[:], lhsT=h2t[:], rhs=wcsb[:], start=True, stop=True
                    )
                    o = hsb.tile([P, NC], dt.float32, tag="o")
                    nc.vector.tensor_tensor(
                        out=o[:], in0=ps3[:], in1=bcsb[:], op=mybir.AluOpType.add
                    )
                    nc.sync.dma_start(out[t * P : t * P + rows, :], o[:rows, :])

        layer(x1[:, :], v1sb, last=False)
        nc.gpsimd.collective_compute(
            "AllGather",
            mybir.AluOpType.bypass,
            replica_groups=[list(range(cfg.n_cores))],
            ins=[h1s.opt()],
            outs=[h1f.opt()],
        )
        layer(h1f[:, :], v2sb, last=True)

    nc.compile()
    return nc


def make_in_maps(cfg: Cfg, per_core, emb, W1, W1_comp, W2, W2_comp, Wc, bc):
    V1 = np.einsum("rb,bio->rio", W1_comp, W1).astype(BF16)
    V2 = np.einsum("rb,bio->rio", W2_comp, W2).astype(BF16)
    x1 = np.ascontiguousarray(np.asarray(emb).astype(BF16))
    wc = np.asarray(Wc).astype(BF16)
    bcb = np.tile(np.asarray(bc).astype(np.float32)[None, :], (P, 1))
    in_maps = []
    for c in range(cfg.n_cores):
        pc = per_core[c]
        in_maps.append(
            dict(
                x1=x1,
                idxp=pc["idx16"],
                dstp=pc["dst_b16"],
                ewp=pc["ew_b16"],
                v1=V1,
                v2=V2,
                wc=wc,
                bcb=bcb,
            )
        )
    return in_maps


def run_program(nc, cfg: Cfg, in_maps, trace=False, tmpdir=None):
    from concourse import bass_utils

    res = bass_utils.run_bass_kernel_spmd(
        nc,
        in_maps,
        core_ids=list(range(cfg.n_cores)),
        trace=trace,
        tmpdir=tmpdir,
    )
    outs = [res.results[c]["out"] for c in range(cfg.n_cores)]
    return np.concatenate(outs, axis=0), res


def kernel(emb, W1, W1_comp, W2, W2_comp, Wc, bc, ew, src, dst):
    cfg = Cfg()
    plan, per_core = preprocess_edges(cfg, src, dst, ew)
    nc = build_program(cfg, plan)
    in_maps = make_in_maps(cfg, per_core, emb, W1, W1_comp, W2, W2_comp, Wc, bc)
    out, _ = run_program(nc, cfg, in_maps, trace=False)
    return out.astype(np.float32)
